# revision 29
# baseline (speedup 1.0000x reference)
"""MoE text projection kernel for 8 TRN2 NeuronCores (Bass/Tile).

Problem: x[32,1024,768], gate_W[768,8], gate_b[8], expert_W[8,768,256],
expert_b[8,256] -> out[32,1024,256].  top-2 of 8 experts, softmax-over-all
gate, dense all-expert projection with masked weighted combine.

Strategy: data-parallel over tokens (32768 tokens -> 4096/core).  Host
pre-transposes x to xT[768, 4096] fp16 per core (contraction dim on
partitions); weights replicated, fp16.  The gate is FUSED into the expert
weight matrix: each contraction chunk carries [8 gate cols][2048 expert
cols], so gate logits come from tiny per-tile N=8 matmuls (~1.5k cycles
total, landing directly in [token, 8] layout) instead of a separate wide
gate pipeline + transpose-back (~25k cycles).  Per 128-token tile:
softmax + top-2 mask via Max8 on VectorE, all-8-expert projections in
fp16 (1 cyc/row) with PSUM accumulation over the 768-contraction,
weighted combine via per-partition-scalar fused multiply-add on VectorE,
expert-bias term via a tiny K=8 matmul (wm^T @ expert_b).  fp16 keeps
rel err ~1.2e-2 (vs 2e-2 budget); fp8/bf16 fail the tolerance.
No collectives: outputs are disjoint token shards, host concatenates.
"""
import sys

sys.path.insert(0, "/opt/trn_rl_repo")

import numpy as np

# hardcoded problem shapes
BS, L, DIN, DOUT, E = 32, 1024, 768, 256, 8
NCORES = 8
NTOK = BS * L              # 32768
T = NTOK // NCORES         # 4096 tokens per core
KC = DIN // 128            # 6 contraction chunks
NG = 8                     # groups per core
TG = T // NG               # 512 tokens per group
NT = TG // 128             # 4 tiles per group

_STATE: dict = {}


def _build_program(reps: int = 1, use_act_round: bool = True,
                   expert_dtype: str = "f32r", dma_engine: str = "sync"):
    import concourse.mybir as mybir
    from concourse import bacc
    from concourse.tile import TileContext
    from concourse.masks import make_identity

    f32 = mybir.dt.float32
    f16 = mybir.dt.float16
    f32r = (mybir.dt.float32r if expert_dtype == "f32r"
            else mybir.dt.bfloat16)

    nc = bacc.Bacc("TRN2", target_bir_lowering=False, debug=False,
                   num_devices=NCORES)
    xT_d = nc.dram_tensor("xt", [DIN, T], f16, kind="ExternalInput")
    gb_d = nc.dram_tensor("gb", [128, NT * E], f32, kind="ExternalInput")
    KW = E * DOUT + E          # per-k-chunk weight cols: 8 gate + 2048 expert
    ew_d = nc.dram_tensor("ew", [128, KC * KW], f16, kind="ExternalInput")
    eb_d = nc.dram_tensor("eb", [E, DOUT], f32, kind="ExternalInput")
    out_d = nc.dram_tensor("out", [T, DOUT], f32, kind="ExternalOutput")

    AL = mybir.AluOpType
    AF = mybir.ActivationFunctionType
    dma = nc.sync if dma_engine == "sync" else nc.gpsimd

    with TileContext(nc) as tc:
        with (
            tc.tile_pool(name="const", bufs=1) as cpool,
            tc.tile_pool(name="xg", bufs=2) as xg_pool,
            tc.tile_pool(name="sm", bufs=4) as sm,
            tc.tile_pool(name="wm", bufs=2) as wm_pool,
            tc.tile_pool(name="wmt", bufs=2) as wmt_pool,
            tc.tile_pool(name="acc", bufs=3) as acc_pool,
            tc.tile_pool(name="pair", bufs=3, space="PSUM") as pair_ps,
            tc.tile_pool(name="gps", bufs=2, space="PSUM") as g_ps,
            tc.tile_pool(name="bps", bufs=1, space="PSUM") as b_ps,
            tc.tile_pool(name="wps", bufs=1, space="PSUM") as w_ps,
        ):
            ident = cpool.tile([128, 128], f32)
            make_identity(nc, ident)
            gb_sb = cpool.tile([128, NT * E], f32)
            eb_sb = cpool.tile([E, DOUT], f32)
            eb_r = cpool.tile([E, DOUT], f32r)
            ew_r = cpool.tile([128, KC * KW], f16)
            dma.dma_start(out=gb_sb, in_=gb_d[:, :])
            dma.dma_start(out=eb_sb, in_=eb_d[:, :])
            nc.vector.tensor_copy(eb_r, eb_sb)

            dma.dma_start(out=ew_r, in_=ew_d[:, :])

            def one_pass():
                for g in range(NG):
                    xg = xg_pool.tile([128, KC * TG], f16, tag="xg")
                    dma.dma_start(
                        out=xg.rearrange("p (k c) -> p k c", k=KC),
                        in_=xT_d.rearrange("(k p) t -> p k t", k=KC, p=128)
                        [:, :, g * TG:(g + 1) * TG],
                    )
                    wm_g = wm_pool.tile([128, NT * E], f32, tag="wmg")
                    wps = w_ps.tile([8, NT * 128], f32, tag="wps")
                    # ---- gate fused into the weight matrix: per-tile tiny
                    # matmuls land logits directly in [token, 8] layout ----
                    lg_g = sm.tile([128, NT * E], f32, tag="lg")
                    ppgs = []
                    for t in range(NT):
                        ppg = g_ps.tile([128, E], f32, tag="ppg",
                                        name=f"ppg{t}")
                        for k in range(KC):
                            nc.tensor.matmul(
                                ppg,
                                xg[:, k * TG + t * 128:
                                   k * TG + (t + 1) * 128],
                                ew_r[:, k * KW:k * KW + E],
                                start=(k == 0), stop=(k == KC - 1),
                            )
                        ppgs.append(ppg)
                    for t in range(NT):
                        nc.vector.tensor_add(
                            lg_g[:, t * E:(t + 1) * E], ppgs[t],
                            gb_sb[:, t * E:(t + 1) * E])
                    ssum_g = sm.tile([128, NT], f32, tag="ssum")
                    rs_g = sm.tile([128, NT], f32, tag="rs")
                    for t in range(NT):
                        lg = lg_g[:, t * E:(t + 1) * E]
                        # ---- softmax + top-2 mask ----
                        m8 = sm.tile([128, 8], f32, tag="m8")
                        nc.vector.max(out=m8, in_=lg)
                        nm1 = sm.tile([128, 1], f32, tag="nm1")
                        nc.vector.tensor_scalar_mul(nm1, m8[:, 0:1], -1.0)
                        keep = sm.tile([128, E], f32, tag="keep")
                        nc.vector.tensor_scalar(
                            keep, lg, m8[:, 1:2], scalar2=None, op0=AL.is_ge)
                        texp = sm.tile([128, E], f32, tag="texp")
                        nc.scalar.activation(
                            texp, lg, AF.Exp, bias=nm1[:, 0:1], scale=1.0,
                            accum_out=ssum_g[:, t:t + 1])
                        # wm_pre = texp * keep (normalize after, batched)
                        nc.vector.tensor_mul(
                            wm_g[:, t * E:(t + 1) * E], texp, keep)
                    nc.vector.reciprocal(rs_g, ssum_g)
                    for t in range(NT):
                        # wm = wm_pre / s
                        nc.vector.tensor_scalar(
                            wm_g[:, t * E:(t + 1) * E],
                            wm_g[:, t * E:(t + 1) * E],
                            rs_g[:, t:t + 1], scalar2=None, op0=AL.mult)

                    acc_g = acc_pool.tile([128, NT * DOUT], f32, tag="acc")

                    def pair_tile(t):
                        acc = acc_g[:, t * DOUT:(t + 1) * DOUT]
                        for pr in range(4):
                            pp = pair_ps.tile([128, 2 * DOUT], f32,
                                              tag="pp", name=f"pp{pr}")
                            for k in range(KC):
                                nc.tensor.matmul(
                                    pp,
                                    xg[:, k * TG + t * 128: k * TG + (t + 1) * 128],
                                    ew_r[:, k * KW + E + 2 * pr * DOUT:
                                         k * KW + E + (2 * pr + 2) * DOUT],
                                    start=(k == 0), stop=(k == KC - 1),
                                )
                            w0 = wm_g[:, t * E + 2 * pr: t * E + 2 * pr + 1]
                            w1 = wm_g[:, t * E + 2 * pr + 1: t * E + 2 * pr + 2]
                            if pr == 0:
                                nc.vector.tensor_scalar(
                                    acc, pp[:, 0:DOUT], w0, scalar2=None,
                                    op0=AL.mult)
                            else:
                                nc.vector.scalar_tensor_tensor(
                                    out=acc, in0=pp[:, 0:DOUT], scalar=w0,
                                    in1=acc, op0=AL.mult, op1=AL.add)
                            nc.vector.scalar_tensor_tensor(
                                out=acc, in0=pp[:, DOUT:2 * DOUT], scalar=w1,
                                in1=acc, op0=AL.mult, op1=AL.add)

                    # tile 0's expert matmuls go first: 12k PE cycles that
                    # hide the softmax latency the wm transposes wait on
                    pair_tile(0)
                    for t in range(NT):
                        nc.tensor.transpose(
                            wps[:, t * 128:(t + 1) * 128],
                            wm_g[:, t * E:(t + 1) * E], ident)
                    wmT_r = wmt_pool.tile([8, NT * 128], f32r, tag="wmt")
                    nc.vector.tensor_copy(wmT_r, wps)
                    bp = b_ps.tile([128, NT * DOUT], f32, tag="bp")
                    for t in range(NT):
                        nc.tensor.matmul(
                            bp[:, t * DOUT:(t + 1) * DOUT],
                            wmT_r[:, t * 128:(t + 1) * 128],
                            eb_r, start=True, stop=True)
                    for t in range(1, NT):
                        pair_tile(t)
                    nc.vector.tensor_add(acc_g, acc_g, bp)
                    dma.dma_start(
                        out=out_d.rearrange("(gg t p) n -> p (gg t) n", p=128, t=NT)
                        [:, g * NT:(g + 1) * NT, :],
                        in_=acc_g.rearrange("p (t n) -> p t n", t=NT),
                    )

            if reps == 1:
                one_pass()
            else:
                with tc.For_i(0, reps, 1):
                    one_pass()

    nc.compile()
    return nc


def _host_prep_weights(gate_W, gate_b, expert_W, expert_b):
    """Rearrange weights into the DMA-friendly layouts (replicated per core)."""
    gate_W = np.asarray(gate_W, dtype=np.float32)
    gate_b = np.asarray(gate_b, dtype=np.float32)
    expert_W = np.asarray(expert_W, dtype=np.float32)
    expert_b = np.asarray(expert_b, dtype=np.float32)
    gb = np.ascontiguousarray(np.tile(gate_b[None, :], (128, NT)))
    # per k-chunk: [8 gate cols][2048 expert cols]
    gwk = gate_W.reshape(KC, 128, E).transpose(1, 0, 2)          # [128,KC,8]
    ewk = (expert_W.reshape(E, KC, 128, DOUT).transpose(2, 1, 0, 3)
           .reshape(128, KC, E * DOUT))                          # [128,KC,2048]
    ew = np.ascontiguousarray(
        np.concatenate([gwk, ewk], axis=2)
        .reshape(128, KC * (E * DOUT + E)).astype(np.float16))
    eb = np.ascontiguousarray(expert_b)
    return gb, ew, eb


def _get_runner(reps: int = 1, **build_kwargs):
    key = ("runner", reps, tuple(sorted(build_kwargs.items())))
    if key in _STATE:
        return _STATE[key]

    import jax
    from jax.sharding import Mesh, PartitionSpec
    from jax.experimental.shard_map import shard_map
    import concourse.mybir as mybir
    from concourse.bass2jax import (
        _bass_exec_p, install_neuronx_cc_hook, partition_id_tensor)

    nc = _build_program(reps=reps, **build_kwargs)
    install_neuronx_cc_hook()

    partition_name = (nc.partition_id_tensor.name
                      if nc.partition_id_tensor else None)
    in_names, out_names, out_avals = [], [], []
    for alloc in nc.m.functions[0].allocations:
        if not isinstance(alloc, mybir.MemoryLocationSet):
            continue
        name = alloc.memorylocations[0].name
        if alloc.kind == "ExternalInput":
            if name != partition_name:
                in_names.append(name)
        elif alloc.kind == "ExternalOutput":
            out_names.append(name)
            out_avals.append(jax.core.ShapedArray(
                tuple(alloc.tensor_shape), mybir.dt.np(alloc.dtype)))
    all_in_names = tuple(in_names) + tuple(out_names)
    if partition_name is not None:
        all_in_names = all_in_names + (partition_name,)
    n_params = len(in_names)

    def _body(*args):
        operands = list(args)
        if partition_name is not None:
            operands.append(partition_id_tensor())
        outs = _bass_exec_p.bind(
            *operands,
            out_avals=tuple(out_avals),
            in_names=all_in_names,
            out_names=tuple(out_names),
            lowering_input_output_aliases=(),
            sim_require_finite=True,
            sim_require_nnan=True,
            nc=nc,
        )
        return tuple(outs)

    devices = jax.devices()[:NCORES]
    mesh = Mesh(np.asarray(devices), ("core",))
    P = PartitionSpec("core")
    n_outs = len(out_names)
    fn = jax.jit(
        shard_map(_body, mesh=mesh,
                  in_specs=(P,) * (n_params + n_outs),
                  out_specs=(P,) * n_outs, check_rep=False),
        donate_argnums=tuple(range(n_params, n_params + n_outs)),
        keep_unused=True,
    )
    runner = {
        "nc": nc, "fn": fn, "in_names": in_names, "out_names": out_names,
        "out_avals": out_avals, "mesh": mesh,
    }
    _STATE[key] = runner
    return runner


def _make_concat_inputs(x, gate_W, gate_b, expert_W, expert_b):
    """Build the concatenated (8*dim0, ...) input arrays in in_names order."""
    x = np.asarray(x, dtype=np.float32)
    gb, ew, eb = _host_prep_weights(gate_W, gate_b, expert_W, expert_b)
    toks = x.reshape(NTOK, DIN).astype(np.float16)
    # per-core transposed shards, stacked: xt_cat[c*DIN:(c+1)*DIN] = shard_c.T
    xt_cat = np.empty((NCORES * DIN, T), np.float16)
    for c in range(NCORES):
        xt_cat[c * DIN:(c + 1) * DIN] = toks[c * T:(c + 1) * T].T
    reps = {
        "xt": xt_cat,
        "gb": np.concatenate([gb] * NCORES, axis=0),
        "ew": np.concatenate([ew] * NCORES, axis=0),
        "eb": np.concatenate([eb] * NCORES, axis=0),
    }
    return reps


def kernel(x, gate_W, gate_b, expert_W, expert_b):
    runner = _get_runner(reps=1)
    cat = _make_concat_inputs(x, gate_W, gate_b, expert_W, expert_b)
    concat_in = [cat[nm] for nm in runner["in_names"]]
    zeros = [np.zeros((NCORES * a.shape[0], *a.shape[1:]), a.dtype)
             for a in runner["out_avals"]]
    outs = runner["fn"](*concat_in, *zeros)
    out_cat = np.asarray(outs[runner["out_names"].index("out")])
    return out_cat.reshape(NCORES * T, DOUT).reshape(BS, L, DOUT)

